# revision 47
# baseline (speedup 1.0000x reference)
"""Trainium2 Bass kernel for nn_Aggregator (BN1d + Swish + Linear + relevance-weighted head sum).

out[b, :] = sum_h w[b,h] * (silu(x[b,h,:] * inv + shift) @ W.T + bias)
          = (sum_h w[b,h] * silu(x[b,h,:] * inv + shift)) @ W.T + (sum_h w[b,h]) * bias

Data parallel over 8 NeuronCores: batch dim B split 8 ways, all params replicated.

Per-core layout (B_loc=1024 b-values -> 8192 flat rows of [512]):
  - 8 "blocks" of 128 b-values; each block = 8 row-tiles of [128 rows, 512].
  - BN affine: DVE mul + DVE/GPSIMD add (params broadcast across partitions).
  - Swish: ScalarE Silu.
  - Weighted head-sum: TensorE matmul with a per-tile staircase matrix
    Wagg[p, b'] = w[row p] * [b' == 16*j + p//8], accumulated over the 8
    row-tiles of a block in PSUM -> g[128 b, 512].
  - g transposed on TensorE (via identity), then g @ W.T on TensorE.
  - bias: out = psum + sumw[b] * bias via one scalar_tensor_tensor.
"""

import os
from contextlib import ExitStack

import numpy as np

import concourse.bacc as bacc
import concourse.mybir as mybir
import concourse.tile as tile
from concourse.bass_utils import run_bass_kernel_spmd
from concourse.mybir import AluOpType

N_CORES = 8
B, H, D, DO = 8192, 8, 512, 256
B_LOC = B // N_CORES            # 1024 b-values per core
ROWS = B_LOC * H                # 8192 flat rows per core
NBLK = B_LOC // 128             # 8 blocks of 128 b-values
EPS = 1e-5
FP = mybir.dt.float32

# Which row-tiles' BN-mul runs on GPSIMD (rest on DVE). Tuned from traces.
GP_MUL_JS = (1, 2, 4, 6)
FPR = mybir.dt.float32r
BF = mybir.dt.bfloat16


ALL_STAGES = frozenset({"bnmul", "bnadd", "silu", "agg", "tail"})


def build_kernel(
    nblk: int = NBLK,
    stages: frozenset = ALL_STAGES,
    bf16_bn: bool = True,
    repeat: int = 1,
):
    """repeat>1 re-runs the whole block loop (same I/O) for slope timing."""
    TB = BF if bf16_bn else FP
    nc = bacc.Bacc(
        "TRN2",
        target_bir_lowering=False,
        debug=False,
        num_devices=N_CORES,
    )

    x_d = nc.dram_tensor("x", (nblk, H, 128, D), FP, kind="ExternalInput")
    # w laid out host-side as [128, ntiles]: column k holds w for flat rows 128k..128k+127
    w_d = nc.dram_tensor("w", (128, nblk * H), FP, kind="ExternalInput")
    sumw_d = nc.dram_tensor("sumw", (128, nblk), FP, kind="ExternalInput")
    invb_d = nc.dram_tensor("invb", (128, D), FP, kind="ExternalInput")
    shiftb_d = nc.dram_tensor("shiftb", (128, D), TB, kind="ExternalInput")
    biasb_d = nc.dram_tensor("biasb", (128, DO), FP, kind="ExternalInput")
    # W.T chunked: wt[p, c*DO + m] = W[m, 128c + p]
    wt_d = nc.dram_tensor("wt", (128, 4 * DO), FPR, kind="ExternalInput")
    # one shared staircase window: astair[p, q] = (q == 112 + p//8);
    # variant j is the slice [:, 112-16j : 240-16j]
    astair_d = nc.dram_tensor("astair", (128, 240), FP, kind="ExternalInput")
    ident_d = nc.dram_tensor("ident", (128, 128), FP, kind="ExternalInput")
    out_d = nc.dram_tensor("out", (nblk, 128, DO), FP, kind="ExternalOutput")

    with tile.TileContext(nc) as tc, ExitStack() as ctx:
        const = ctx.enter_context(tc.tile_pool(name="const", bufs=1))
        xpool = ctx.enter_context(tc.tile_pool(name="xin", bufs=5))
        # DMA issue order tuned for fill: first x slices, then the small
        # tiles the first row-tiles depend on, then the rest of block 0,
        # then the bulkier tail constants.
        xt0 = xpool.tile([128, H * D], FP, tag="xt")
        for j in range(2):
            nc.sync.dma_start(xt0[:, j * D : (j + 1) * D], x_d.ap()[0][j])
        invb = const.tile([128, D], FP)
        nc.sync.dma_start(invb[:], invb_d.ap())
        shiftb = const.tile([128, D], TB)
        nc.sync.dma_start(shiftb[:], shiftb_d.ap())
        astair = const.tile([128, 240], FP)
        nc.sync.dma_start(astair[:], astair_d.ap())
        w_sb = const.tile([128, nblk * H], FP)
        nc.sync.dma_start(w_sb[:], w_d.ap())
        for j in range(2, H):
            nc.sync.dma_start(xt0[:, j * D : (j + 1) * D], x_d.ap()[0][j])

        biasb = const.tile([128, DO], FP)
        nc.sync.dma_start(biasb[:], biasb_d.ap())
        wt = const.tile([128, 4 * DO], FPR)
        nc.sync.dma_start(wt[:], wt_d.ap())
        ident = const.tile([128, 128], FP)
        nc.sync.dma_start(ident[:], ident_d.ap())
        sumw = const.tile([128, nblk], FP)
        nc.sync.dma_start(sumw[:], sumw_d.ap())
        tpool = ctx.enter_context(tc.tile_pool(name="tmp", bufs=10))
        spool = ctx.enter_context(tc.tile_pool(name="act", bufs=6))
        wgpool = ctx.enter_context(tc.tile_pool(name="wagg", bufs=6))
        gpool = ctx.enter_context(tc.tile_pool(name="g", bufs=2))
        gtpool = ctx.enter_context(tc.tile_pool(name="gt", bufs=2))
        opool = ctx.enter_context(tc.tile_pool(name="o", bufs=2))
        psg = ctx.enter_context(tc.tile_pool(name="psg", bufs=3, space="PSUM"))
        pst = ctx.enter_context(tc.tile_pool(name="pst", bufs=2, space="PSUM"))
        pso = ctx.enter_context(tc.tile_pool(name="pso", bufs=2, space="PSUM"))

        for rep in range(repeat):
          for n in range(nblk):
            if n == 0 and rep == 0:
                xt = xt0
            else:
                # split loads: halves mid-stream, quarters for the last block
                # so its compute chain starts as early as possible
                xt = xpool.tile([128, H * D], FP, tag="xt")
                nsplit = 4 if n == nblk - 1 else 2
                js_per = H // nsplit
                for sp in range(nsplit):
                    j0 = sp * js_per
                    nc.sync.dma_start(
                        xt[:, j0 * D : (j0 + js_per) * D].rearrange(
                            "p (j d) -> p j d", j=js_per
                        ),
                        x_d.ap()[n][j0 : j0 + js_per].rearrange("j p d -> p j d"),
                    )
            pg = psg.tile([128, D], FP)
            for k in range(H // 2):
                # pair of row-tiles j=2k, 2k+1 processed at [128, 2*D]
                t1 = tpool.tile([128, 2 * D], TB, tag="t1")
                for v in range(2):
                    j = 2 * k + v
                    if "bnmul" in stages:
                        # late blocks lean DVE so the GP doesn't pace the tail
                        if n == nblk - 1:
                            on_gp = j in (0, 2)
                        elif n == nblk - 2:
                            on_gp = j in (0, 2, 4, 6)
                        else:
                            on_gp = j in GP_MUL_JS
                        eng = nc.gpsimd if on_gp else nc.vector
                        eng.tensor_tensor(
                            t1[:, v * D : (v + 1) * D],
                            xt[:, j * D : (j + 1) * D],
                            invb[:],
                            AluOpType.mult,
                        )
                cur = t1[:]
                if "bnadd" in stages:
                    t2 = tpool.tile([128, 2 * D], TB, tag="t2")
                    nc.vector.tensor_tensor(
                        t2[:].rearrange("p (r d) -> p r d", r=2),
                        cur.rearrange("p (r d) -> p r d", r=2),
                        shiftb[:].unsqueeze(1).broadcast_to([128, 2, D]),
                        AluOpType.add,
                    )
                    cur = t2[:]
                if "silu" in stages:
                    s = spool.tile([128, 2 * D], FPR)
                    nc.scalar.activation(
                        s[:], cur, mybir.ActivationFunctionType.Silu
                    )
                    cur = s[:]
                if "agg" in stages:
                    # Wagg[p, c] = w[row p] * (c == 16*j + p//8); all 8
                    # row-tiles of the block accumulate into pg[:, :].
                    for v in range(2):
                        j = 2 * k + v
                        wg = wgpool.tile([128, 128], FPR)
                        wg_eng = nc.gpsimd if v == 0 else nc.vector
                        wg_eng.tensor_scalar_mul(
                            wg[:],
                            astair[:, 112 - 16 * j : 240 - 16 * j],
                            w_sb[:, n * H + j : n * H + j + 1],
                        )
                        nc.tensor.matmul(
                            pg[:],
                            wg[:],
                            cur[:, v * D : (v + 1) * D],
                            start=(j == 0),
                            stop=(j == H - 1),
                        )
            if "tail" not in stages:
                continue
            g = gpool.tile([128, D], FP)
            nc.scalar.copy(g[:], pg[:])
            pt = pst.tile([128, D], FP)
            for c in range(4):
                nc.tensor.transpose(
                    pt[:, c * 128 : (c + 1) * 128],
                    g[:, c * 128 : (c + 1) * 128],
                    ident[:],
                )
            gt = gtpool.tile([128, D], FPR)
            nc.scalar.copy(gt[:], pt[:])
            po = pso.tile([128, DO], FP)
            for c in range(4):
                nc.tensor.matmul(
                    po[:],
                    gt[:, c * 128 : (c + 1) * 128],
                    wt[:, c * DO : (c + 1) * DO],
                    start=(c == 0),
                    stop=(c == 3),
                )
            o = opool.tile([128, DO], FP)
            nc.vector.scalar_tensor_tensor(
                o[:], biasb[:], sumw[:, n : n + 1], po[:],
                AluOpType.mult, AluOpType.add,
            )
            # stores go out the ACT HWDGE queue so a store waiting on compute
            # never blocks the next x load in the SP queue's FIFO
            nc.scalar.dma_start(out_d.ap()[n], o[:])

    nc.compile()
    return nc


def make_host_inputs(x_np, w_np, gamma, beta, mean, var, W, b, nblk: int = NBLK, bf16_bn: bool = True):
    """Build the per-core input maps (host-side layout prep only)."""
    import ml_dtypes

    inv = (gamma / np.sqrt(var + EPS)).astype(np.float32)
    shift = (beta - mean * inv).astype(np.float32)
    invb = np.ascontiguousarray(np.broadcast_to(inv, (128, D)))
    sdt = ml_dtypes.bfloat16 if bf16_bn else np.float32
    shiftb = np.ascontiguousarray(np.broadcast_to(shift.astype(sdt), (128, D)))
    biasb = np.ascontiguousarray(np.broadcast_to(b.astype(np.float32), (128, DO)))
    wt = np.ascontiguousarray(
        W.astype(np.float32).T.reshape(4, 128, DO).transpose(1, 0, 2).reshape(128, 4 * DO)
    )
    p = np.arange(128)
    astair = np.zeros((128, 240), dtype=np.float32)
    astair[p, 112 + p // 8] = 1.0
    ident = np.eye(128, dtype=np.float32)

    rows_loc = nblk * H * 128
    b_loc = nblk * 128
    in_maps = []
    for core in range(N_CORES):
        b0 = core * B_LOC
        # flat row order: row = ((n*H + j)*128 + p) -> x tile [n, j, p, d]
        x_loc = np.ascontiguousarray(x_np[b0 : b0 + b_loc].reshape(nblk, H, 128, D))
        w_flat = w_np[b0 : b0 + b_loc].reshape(rows_loc).astype(np.float32)
        w_sb = np.ascontiguousarray(w_flat.reshape(nblk * H, 128).T)
        sumw = w_np[b0 : b0 + b_loc].sum(axis=1).astype(np.float32)
        sumw_sb = np.ascontiguousarray(sumw.reshape(nblk, 128).T)
        in_maps.append(
            {
                "x": x_loc,
                "w": w_sb,
                "sumw": sumw_sb,
                "invb": invb,
                "shiftb": shiftb,
                "biasb": biasb,
                "wt": wt,
                "astair": astair,
                "ident": ident,
            }
        )
    return in_maps


_NC_CACHE = None
LAST_RESULT = None


def make_runner(nc, in_maps):
    """Build a reusable jitted SPMD callable with device-resident inputs.

    Mirrors bass2jax.run_bass_via_pjrt's multi-core path, but without
    donation so the same device buffers can be executed repeatedly for
    steady-state timing.
    """
    import jax
    from concourse import bass2jax
    from jax.experimental.shard_map import shard_map
    from jax.sharding import Mesh, NamedSharding, PartitionSpec

    bass2jax.install_neuronx_cc_hook()
    partition_name = nc.partition_id_tensor.name if nc.partition_id_tensor else None
    in_names, out_names, out_avals, zero_outs = [], [], [], []
    for alloc in nc.m.functions[0].allocations:
        if not isinstance(alloc, mybir.MemoryLocationSet):
            continue
        name = alloc.memorylocations[0].name
        if alloc.kind == "ExternalInput":
            if name != partition_name:
                in_names.append(name)
        elif alloc.kind == "ExternalOutput":
            out_names.append(name)
            shape = tuple(alloc.tensor_shape)
            dtype = mybir.dt.np(alloc.dtype)
            out_avals.append(jax.core.ShapedArray(shape, dtype))
            zero_outs.append(np.zeros(shape, dtype))
    n_params = len(in_names)
    all_names = in_names + out_names
    if partition_name is not None:
        all_names = all_names + [partition_name]

    def _body(*args):
        operands = list(args)
        if partition_name is not None:
            operands.append(bass2jax.partition_id_tensor())
        outs = bass2jax._bass_exec_p.bind(
            *operands,
            out_avals=tuple(out_avals),
            in_names=tuple(all_names),
            out_names=tuple(out_names),
            lowering_input_output_aliases=(),
            sim_require_finite=True,
            sim_require_nnan=True,
            nc=nc,
        )
        return tuple(outs)

    n_cores = len(in_maps)
    devices = jax.devices()[:n_cores]
    mesh = Mesh(np.asarray(devices), ("core",))
    in_specs = (PartitionSpec("core"),) * (n_params + len(out_names))
    out_specs = (PartitionSpec("core"),) * len(out_names)
    fn = jax.jit(
        shard_map(_body, mesh=mesh, in_specs=in_specs, out_specs=out_specs,
                  check_rep=False),
        keep_unused=True,
    )
    sh = NamedSharding(mesh, PartitionSpec("core"))
    concat = [
        np.concatenate([np.asarray(m[name]) for m in in_maps], axis=0)
        for name in in_names
    ] + [np.zeros((n_cores * z.shape[0], *z.shape[1:]), z.dtype) for z in zero_outs]
    dev_in = [jax.device_put(a, sh) for a in concat]
    return fn, dev_in, out_names, out_avals


def kernel(
    x_concepts_encoded, relevance_weights, bn_gamma, bn_beta, bn_mean, bn_var, W, b
):
    global _NC_CACHE, LAST_RESULT
    x_np = np.asarray(x_concepts_encoded, dtype=np.float32)
    w_np = np.asarray(relevance_weights, dtype=np.float32)
    if _NC_CACHE is None:
        _NC_CACHE = build_kernel()
    nc = _NC_CACHE
    in_maps = make_host_inputs(
        x_np,
        w_np,
        np.asarray(bn_gamma, dtype=np.float32),
        np.asarray(bn_beta, dtype=np.float32),
        np.asarray(bn_mean, dtype=np.float32),
        np.asarray(bn_var, dtype=np.float32),
        np.asarray(W, dtype=np.float32),
        np.asarray(b, dtype=np.float32),
    )
    trace = bool(int(os.environ.get("KERNEL_TRACE", "0")))
    LAST_RESULT = run_bass_kernel_spmd(
        nc, in_maps, core_ids=list(range(N_CORES)), trace=trace
    )
    out = np.concatenate(
        [LAST_RESULT.results[i]["out"].reshape(B_LOC, DO) for i in range(N_CORES)],
        axis=0,
    )
    return out
